# revision 1
# baseline (speedup 1.0000x reference)
"""Trainium2 Bass kernel for nn_ContinuousConvolutionBlock (gnn_message_passing).

Strategy (per sharding hint: partition points across 8 cores; each core owns its
queries' scatter-reduce and tap-GEMM; filter + dense weights replicated):

Host side (index plumbing / input marshalling only — zero FLOPs):
  - qry_idx is sorted; queries are grouped into 8-query blocks, blocks paired
    into 128-edge-slot "chunks" (two-pointer bin packing, ~3% padding).
  - Consecutive block ranges are assigned to the 8 cores; per-core per-slot
    payload arrays (pos[src], pos[qry], feats[src], local query id) are
    marshalled on host and DMA'd in dense [128 x NCH x k] layout.

Device side (all FLOP-bearing compute):
  - Geometry: ball->cube volume-preserving map + trilinear corner weights
    (DVE arithmetic + ACT sqrt/arctan/sign/abs), producing per-slot 4-wide
    one-hot weight vectors w4x/w4y/w4z (separable trilinear factorization).
  - Scatter-reduce as factored matmul per chunk: with R[slot,(ax,c)] =
    w4x (x) feats and L[slot,(q,az,ay)] = Qoh (x) w4z (x) w4y, PE computes
    A^T[(ax,c),(q,az,ay)] = R^T @ L, accumulating the per-query tap grid
    A[q, az,ay,ax, c] directly in transposed layout (PSUM).
  - Tap-GEMM: for each (az,ay) tap-pair t, out^T += G_t^T @ A^T-slices,
    accumulated over 16 t in PSUM. G is the filter regrouped on host
    (pure relayout, replicated to all cores).
  - Dense branch: out_dense^T = dense_w^T @ feats^T + b on PE.
  Outputs are produced transposed ([64, nq]); host transposes/reorders back.
"""
import sys
import os
sys.path.insert(0, '/opt/trn_rl_repo')
import numpy as np

N = 30000
CIN = 32
COUT = 64
KS = 4
EXTENT = 0.08
NCORES = 8
NBLK = N // 8  # 3750 eight-query blocks

_COMPILED = {}


# ----------------------------------------------------------------------------
# Host planning
# ----------------------------------------------------------------------------
def _plan(qry_idx):
    deg = np.bincount(qry_idx, minlength=N)
    bsz = deg.reshape(NBLK, 8).sum(1)
    bstart = np.concatenate([[0], np.cumsum(bsz)]).astype(np.int64)
    per = [NBLK // NCORES + (1 if c < NBLK % NCORES else 0) for c in range(NCORES)]
    b0 = np.concatenate([[0], np.cumsum(per)]).astype(np.int64)
    plans = []
    for c in range(NCORES):
        blocks = list(range(b0[c], b0[c + 1]))
        asc = sorted(blocks, key=lambda b: bsz[b])
        chunks = []
        lo, hi = 0, len(asc) - 1
        while lo <= hi:
            if lo == hi:
                chunks.append((asc[hi], None)); break
            if bsz[asc[hi]] + bsz[asc[lo]] <= 128:
                chunks.append((asc[hi], asc[lo])); hi -= 1; lo += 1
            else:
                chunks.append((asc[hi], None)); hi -= 1
        plans.append(dict(blocks=blocks, chunks=chunks, q0=int(8 * b0[c]),
                          nq=int(8 * (b0[c + 1] - b0[c]))))
    return plans, bstart, bsz


def _pack_core(plan_c, bstart, pos, feats, qry_idx, src_idx, NCHP):
    """Build per-slot payload arrays in [128, NCHP, k] layout."""
    possrc = np.zeros((128, NCHP, 4), np.float32)
    posqry = np.zeros((128, NCHP, 4), np.float32)
    fsrc = np.zeros((128, NCHP, CIN), np.float32)
    qlocf = np.full((128, NCHP), -1.0, np.float32)
    for ci, (bA, bB) in enumerate(plan_c['chunks']):
        s = 0
        for half, b in enumerate((bA, bB)):
            if b is None:
                continue
            e0, e1 = int(bstart[b]), int(bstart[b + 1])
            n = e1 - e0
            sl = slice(s, s + n)
            possrc[sl, ci, 0:3] = pos[src_idx[e0:e1]]
            posqry[sl, ci, 0:3] = pos[qry_idx[e0:e1]]
            fsrc[sl, ci, :] = feats[src_idx[e0:e1]]
            qlocf[sl, ci] = (qry_idx[e0:e1] - 8 * b) + 8 * half
            s += n
    return possrc, posqry, fsrc, qlocf


# ----------------------------------------------------------------------------
# Device kernel
# ----------------------------------------------------------------------------
def _build_bass(NCHP, NQ):
    import concourse.bass as bass
    import concourse.tile as tile
    from concourse import bacc, mybir
    from concourse.bass import AP

    f32 = mybir.dt.float32
    f32r = mybir.dt.float32r
    i32 = mybir.dt.int32
    ALU = mybir.AluOpType
    ACT = mybir.ActivationFunctionType
    EPS = 1e-12
    F4PI = float(4.0 / np.pi)

    nc = bacc.Bacc("TRN2", target_bir_lowering=False, debug=False)

    possrc = nc.dram_tensor("possrc", (128, NCHP, 4), f32, kind="ExternalInput")
    posqry = nc.dram_tensor("posqry", (128, NCHP, 4), f32, kind="ExternalInput")
    fsrc = nc.dram_tensor("fsrc", (128, NCHP, CIN), f32, kind="ExternalInput")
    qlocf = nc.dram_tensor("qlocf", (128, NCHP), f32, kind="ExternalInput")
    g2 = nc.dram_tensor("g2", (128, 16 * 64), f32, kind="ExternalInput")
    featsT = nc.dram_tensor("featsT", (CIN, NQ), f32, kind="ExternalInput")
    denw = nc.dram_tensor("denw", (CIN, COUT), f32, kind="ExternalInput")
    denb = nc.dram_tensor("denb", (COUT, 1), f32, kind="ExternalInput")

    outconvT = nc.dram_tensor("outconvT", (COUT, NQ), f32, kind="ExternalOutput")
    outdenseT = nc.dram_tensor("outdenseT", (COUT, NQ), f32, kind="ExternalOutput")

    W = NCHP            # geometry tile width (all chunks at once)
    NGRP = NCHP // 16   # tap-GEMM groups

    with tile.TileContext(nc) as tc:
        with tc.tile_pool(name="inp", bufs=1) as inp, \
             tc.tile_pool(name="geo", bufs=1) as geo, \
             tc.tile_pool(name="tmp", bufs=1) as tmp, \
             tc.tile_pool(name="lr", bufs=10) as lrp, \
             tc.tile_pool(name="at", bufs=3) as atp, \
             tc.tile_pool(name="outp", bufs=4) as outp, \
             tc.tile_pool(name="ps1", bufs=4, space="PSUM") as ps1, \
             tc.tile_pool(name="ps2", bufs=2, space="PSUM") as ps2:

            # ---------------- input DMAs ----------------
            t_ps = inp.tile([128, W, 4], f32)
            t_pq = inp.tile([128, W, 4], f32)
            t_f = inp.tile([128, W, CIN], f32)
            t_ql = inp.tile([128, W], f32)
            t_g2 = inp.tile([128, 16 * 64], f32)
            t_ftT = inp.tile([CIN, NQ], f32)
            t_dw = inp.tile([CIN, COUT], f32)
            t_db = inp.tile([COUT, 1], f32)
            nc.sync.dma_start(t_ps[:], possrc[:])
            nc.sync.dma_start(t_pq[:], posqry[:])
            nc.sync.dma_start(t_f[:], fsrc[:])
            nc.sync.dma_start(t_ql[:], qlocf[:])
            nc.sync.dma_start(t_g2[:], g2[:])
            nc.sync.dma_start(t_ftT[:], featsT[:])
            nc.sync.dma_start(t_dw[:], denw[:])
            nc.sync.dma_start(t_db[:], denb[:])

            # round filter to f32r once
            t_g2r = inp.tile([128, 16 * 64], f32r)
            nc.vector.tensor_copy(t_g2r[:], t_g2[:])

            # iota constants
            io4i = tmp.tile([128, 4], i32)
            nc.gpsimd.iota(io4i[:], pattern=[[1, 4]], base=0, channel_multiplier=0)
            io4 = geo.tile([128, 4], f32)
            nc.vector.tensor_copy(io4[:], io4i[:])
            io16i = tmp.tile([128, 16], i32)
            nc.gpsimd.iota(io16i[:], pattern=[[1, 16]], base=0, channel_multiplier=0)
            io16 = geo.tile([128, 16], f32)
            nc.vector.tensor_copy(io16[:], io16i[:])

            # ---------------- geometry ----------------
            _tn = [0]
            _free_tags = []
            _tag_of = {}

            _seq = [0]

            def T(shape=(128, W), dt_=f32):
                if _free_tags:
                    tg = _free_tags.pop()
                else:
                    _tn[0] += 1
                    tg = f"t{_tn[0]}"
                _seq[0] += 1
                t = tmp.tile(list(shape), dt_, name=f"{tg}_u{_seq[0]}", tag=tg)
                _tag_of[id(t)] = tg
                return t

            def F(*ts):
                for t in ts:
                    _free_tags.append(_tag_of.pop(id(t)))

            TT = nc.vector.tensor_tensor
            TS = nc.vector.tensor_scalar
            STT = nc.vector.scalar_tensor_tensor

            # r = (ps - pq) * (2/EXTENT), per coord [128, W, 3]
            r = T((128, W, 3))
            TT(out=r[:], in0=t_ps[:, :, 0:3], in1=t_pq[:, :, 0:3], op=ALU.subtract)
            rs = T((128, W, 3))
            TS(rs[:], r[:], float(2.0 / EXTENT), None, op0=ALU.mult)
            F(r)
            x, y, z = rs[:, :, 0], rs[:, :, 1], rs[:, :, 2]

            sq3 = T((128, W, 3))
            TT(out=sq3[:], in0=rs[:], in1=rs[:], op=ALU.mult)
            x2, y2, z2 = sq3[:, :, 0], sq3[:, :, 1], sq3[:, :, 2]
            xy2 = T()
            TT(out=xy2[:], in0=x2, in1=y2, op=ALU.add)
            sq = T()
            TT(out=sq[:], in0=xy2[:], in1=z2, op=ALU.add)

            norm = T()
            nc.scalar.activation(norm[:], sq[:], ACT.Sqrt)
            nxy = T()
            nc.scalar.activation(nxy[:], xy2[:], ACT.Sqrt)

            p125 = T()
            TS(p125[:], z2, 1.25, None, op0=ALU.mult)
            pole = T()
            TT(out=pole[:], in0=p125[:], in1=xy2[:], op=ALU.is_gt)
            F(sq3, xy2, p125)

            azn = T()
            nc.scalar.activation(azn[:], z, ACT.Abs)
            den1 = T()
            STT(out=den1[:], in0=azn[:], scalar=EPS, in1=norm[:], op0=ALU.add, op1=ALU.add)
            rd1 = T()
            nc.vector.reciprocal(rd1[:], den1[:])
            t1s = T()
            STT(out=t1s[:], in0=norm[:], scalar=3.0, in1=rd1[:], op0=ALU.mult, op1=ALU.mult)
            s1 = T()
            nc.scalar.activation(s1[:], t1s[:], ACT.Sqrt)
            F(azn, den1, rd1, t1s)

            den2 = T()
            TS(den2[:], nxy[:], EPS, None, op0=ALU.add)
            rd2 = T()
            nc.vector.reciprocal(rd2[:], den2[:])
            s2 = T()
            TT(out=s2[:], in0=norm[:], in1=rd2[:], op=ALU.mult)
            F(nxy, den2, rd2)

            d12 = T()
            TT(out=d12[:], in0=s1[:], in1=s2[:], op=ALU.subtract)
            pw = T()
            TT(out=pw[:], in0=pole[:], in1=d12[:], op=ALU.mult)
            wq = T()
            TT(out=wq[:], in0=s2[:], in1=pw[:], op=ALU.add)
            F(s1, s2, d12, pw)

            xc = T()
            TT(out=xc[:], in0=x, in1=wq[:], op=ALU.mult)
            yc = T()
            TT(out=yc[:], in0=y, in1=wq[:], op=ALU.mult)
            F(wq)

            sgz = T()
            nc.scalar.activation(sgz[:], z, ACT.Sign)
            zcp = T()
            TT(out=zcp[:], in0=sgz[:], in1=norm[:], op=ALU.mult)
            zce = T()
            TS(zce[:], z, 1.5, None, op0=ALU.mult)
            dz = T()
            TT(out=dz[:], in0=zcp[:], in1=zce[:], op=ALU.subtract)
            pz = T()
            TT(out=pz[:], in0=pole[:], in1=dz[:], op=ALU.mult)
            zc = T()
            TT(out=zc[:], in0=zce[:], in1=pz[:], op=ALU.add)
            F(sgz, zcp, zce, dz, pz, pole, norm, rs)

            zero1 = T()
            TS(zero1[:], sq[:], EPS, None, op0=ALU.is_lt)
            onem1 = T()
            TS(onem1[:], zero1[:], -1.0, 1.0, op0=ALU.mult, op1=ALU.add)
            for t_ in (xc, yc, zc):
                TT(out=t_[:], in0=t_[:], in1=onem1[:], op=ALU.mult)
            F(sq, zero1, onem1)

            # cylinder -> cube
            xc2 = T()
            TT(out=xc2[:], in0=xc[:], in1=xc[:], op=ALU.mult)
            yc2 = T()
            TT(out=yc2[:], in0=yc[:], in1=yc[:], op=ALU.mult)
            sqxy = T()
            TT(out=sqxy[:], in0=xc2[:], in1=yc2[:], op=ALU.add)
            nrm = T()
            nc.scalar.activation(nrm[:], sqxy[:], ACT.Sqrt)
            F(xc2, yc2)

            axc = T()
            nc.scalar.activation(axc[:], xc[:], ACT.Abs)
            ayc = T()
            nc.scalar.activation(ayc[:], yc[:], ACT.Abs)
            abr = T()
            TT(out=abr[:], in0=ayc[:], in1=axc[:], op=ALU.is_le)

            mx = T()
            TS(mx[:], axc[:], EPS, None, op0=ALU.is_lt)
            sfx = T()
            TT(out=sfx[:], in0=xc[:], in1=mx[:], op=ALU.add)
            my = T()
            TS(my[:], ayc[:], EPS, None, op0=ALU.is_lt)
            sfy = T()
            TT(out=sfy[:], in0=yc[:], in1=my[:], op=ALU.add)
            F(axc, ayc, mx, my)

            rsx = T()
            nc.vector.reciprocal(rsx[:], sfx[:])
            rsy = T()
            nc.vector.reciprocal(rsy[:], sfy[:])
            ratx = T()
            TT(out=ratx[:], in0=xc[:], in1=rsy[:], op=ALU.mult)
            raty = T()
            TT(out=raty[:], in0=yc[:], in1=rsx[:], op=ALU.mult)
            at1 = T()
            nc.scalar.activation(at1[:], ratx[:], ACT.Arctan)
            at2 = T()
            nc.scalar.activation(at2[:], raty[:], ACT.Arctan)
            F(sfx, sfy, rsx, rsy, ratx, raty)

            sgx = T()
            nc.scalar.activation(sgx[:], xc[:], ACT.Sign)
            sgy = T()
            nc.scalar.activation(sgy[:], yc[:], ACT.Sign)
            tmpa = T()
            TT(out=tmpa[:], in0=sgx[:], in1=nrm[:], op=ALU.mult)
            tmpb = T()
            TT(out=tmpb[:], in0=sgy[:], in1=nrm[:], op=ALU.mult)
            F(sgx, sgy, nrm, xc, yc)

            # xo = where(a, tmpa, tmpb * F4PI * at1)
            xoe = T()
            STT(out=xoe[:], in0=at1[:], scalar=F4PI, in1=tmpb[:], op0=ALU.mult, op1=ALU.mult)
            dxo = T()
            TT(out=dxo[:], in0=tmpa[:], in1=xoe[:], op=ALU.subtract)
            adx = T()
            TT(out=adx[:], in0=abr[:], in1=dxo[:], op=ALU.mult)
            xo = T()
            TT(out=xo[:], in0=xoe[:], in1=adx[:], op=ALU.add)
            # yo = where(a, tmpa * F4PI * at2, tmpb)
            yoe = T()
            STT(out=yoe[:], in0=at2[:], scalar=F4PI, in1=tmpa[:], op0=ALU.mult, op1=ALU.mult)
            dyo = T()
            TT(out=dyo[:], in0=yoe[:], in1=tmpb[:], op=ALU.subtract)
            ady = T()
            TT(out=ady[:], in0=abr[:], in1=dyo[:], op=ALU.mult)
            yo = T()
            TT(out=yo[:], in0=tmpb[:], in1=ady[:], op=ALU.add)
            F(at1, at2, xoe, dxo, adx, yoe, dyo, ady, tmpa, tmpb, abr)

            zero2 = T()
            TS(zero2[:], sqxy[:], EPS, None, op0=ALU.is_lt)
            onem2 = T()
            TS(onem2[:], zero2[:], -1.0, 1.0, op0=ALU.mult, op1=ALU.add)
            TT(out=xo[:], in0=xo[:], in1=onem2[:], op=ALU.mult)
            TT(out=yo[:], in0=yo[:], in1=onem2[:], op=ALU.mult)
            F(sqxy, zero2, onem2)

            # ---------------- corner weights w4 ----------------
            def corners_w4(m_ap, w4_t):
                g = T()
                TS(g[:], m_ap, 1.5, 1.5, op0=ALU.mult, op1=ALU.add)
                gc = T()
                TS(gc[:], g[:], 0.0, None, op0=ALU.max)
                g0i = T(dt_=i32)
                TS(g0i[:], gc[:], 0.5, None, op0=ALU.subtract)  # cast rint => floor
                g0 = T()
                nc.vector.tensor_copy(g0[:], g0i[:])
                fr = T()
                TT(out=fr[:], in0=gc[:], in1=g0[:], op=ALU.subtract)
                i0 = T()
                TS(i0[:], g0[:], 3.0, None, op0=ALU.min)
                i1 = T()
                TS(i1[:], g0[:], 1.0, 3.0, op0=ALU.add, op1=ALU.min)
                # e0/e1 one-hots [128, W, 4]
                e0 = T((128, W, 4))
                TT(out=e0[:],
                   in0=AP(io4.tensor, io4[:].offset, [io4[:].ap[0], [0, W], [1, 4]]),
                   in1=AP(i0.tensor, i0[:].offset, [i0[:].ap[0], [1, W], [0, 4]]),
                   op=ALU.is_equal)
                e1 = T((128, W, 4))
                TT(out=e1[:],
                   in0=AP(io4.tensor, io4[:].offset, [io4[:].ap[0], [0, W], [1, 4]]),
                   in1=AP(i1.tensor, i1[:].offset, [i1[:].ap[0], [1, W], [0, 4]]),
                   op=ALU.is_equal)
                onemf = T()
                TS(onemf[:], fr[:], -1.0, 1.0, op0=ALU.mult, op1=ALU.add)
                TT(out=e0[:], in0=e0[:],
                   in1=AP(onemf.tensor, onemf[:].offset, [onemf[:].ap[0], [1, W], [0, 4]]),
                   op=ALU.mult)
                TT(out=e1[:], in0=e1[:],
                   in1=AP(fr.tensor, fr[:].offset, [fr[:].ap[0], [1, W], [0, 4]]),
                   op=ALU.mult)
                TT(out=w4_t[:], in0=e0[:], in1=e1[:], op=ALU.add)
                F(g, gc, g0i, g0, fr, i0, i1, e0, e1, onemf)

            w4x = geo.tile([128, W, 4], f32)
            w4y = geo.tile([128, W, 4], f32)
            w4z = geo.tile([128, W, 4], f32)
            corners_w4(xo[:], w4x)
            corners_w4(yo[:], w4y)
            corners_w4(zc[:], w4z)
            F(xo, yo, zc)

            # Qoh16 [128, W, 16], ZY [128, W, 16]
            qoh = geo.tile([128, W, 16], f32)
            TT(out=qoh[:],
               in0=AP(t_ql.tensor, t_ql[:].offset, [t_ql[:].ap[0], [1, W], [0, 16]]),
               in1=AP(io16.tensor, io16[:].offset, [io16[:].ap[0], [0, W], [1, 16]]),
               op=ALU.is_equal)
            zy = geo.tile([128, W, 16], f32)
            TT(out=zy[:],
               in0=AP(w4z.tensor, w4z[:].offset,
                      [w4z[:].ap[0], [4, W], [1, 4], [0, 4]]),
               in1=AP(w4y.tensor, w4y[:].offset,
                      [w4y[:].ap[0], [4, W], [0, 4], [1, 4]]),
               op=ALU.mult)

            # ---------------- stage-1 + tap-GEMM ----------------
            for g in range(NGRP):
                at_st = atp.tile([128, 16 * 256], f32r, tag="at")
                for cl in range(0, 16, 2):
                    ps_t = ps1.tile([128, 512], f32, space="PSUM", tag="s1")
                    for par in range(2):
                        ci = g * 16 + cl + par
                        # R [128, (ax, c)]
                        R = lrp.tile([128, 128], f32r, tag="R")
                        wx = w4x[:, ci, :]
                        ff = t_f[:, ci, :]
                        TT(out=AP(R.tensor, R[:].offset, [R[:].ap[0], [32, 4], [1, 32]]),
                           in0=AP(wx.tensor, wx.offset, [wx.ap[0], [1, 4], [0, 32]]),
                           in1=AP(ff.tensor, ff.offset, [ff.ap[0], [0, 4], [1, 32]]),
                           op=ALU.mult)
                        # L [128, (half, q, t)]
                        L = lrp.tile([128, 256], f32r, tag="L")
                        qq = qoh[:, ci, :]
                        zz = zy[:, ci, :]
                        TT(out=AP(L.tensor, L[:].offset,
                                  [L[:].ap[0], [128, 2], [16, 8], [1, 16]]),
                           in0=AP(qq.tensor, qq.offset,
                                  [qq.ap[0], [8, 2], [1, 8], [0, 16]]),
                           in1=AP(zz.tensor, zz.offset,
                                  [zz.ap[0], [0, 2], [0, 8], [1, 16]]),
                           op=ALU.mult)
                        nc.tensor.matmul(
                            out=ps_t[:, par * 256:(par + 1) * 256],
                            lhsT=R[:], rhs=L[:], start=True, stop=True)
                    # copy 2 chunks at once, alternating DVE/ACT
                    dst = at_st[:, cl * 256:(cl + 2) * 256]
                    if (cl // 2) % 2 == 0:
                        nc.vector.tensor_copy(dst, ps_t[:])
                    else:
                        nc.scalar.copy(dst, ps_t[:])
                # tap-GEMM for this group
                po = ps2.tile([COUT, 256], f32, space="PSUM", tag="tap")
                for t in range(16):
                    rhs = AP(at_st.tensor, at_st[:].offset + t,
                             [at_st[:].ap[0], [256, 16], [128, 2], [16, 8]])
                    nc.tensor.matmul(
                        out=po[:],
                        lhsT=t_g2r[:, t * 64:(t + 1) * 64],
                        rhs=rhs,
                        start=(t == 0), stop=(t == 15))
                ost = outp.tile([COUT, 256], f32, tag="ocst")
                nc.vector.tensor_copy(ost[:], po[:])
                nc.sync.dma_start(outconvT[:, g * 256:(g + 1) * 256], ost[:])

            # ---------------- dense branch (plain fp32 matmul) ----------------
            NSEG = (NQ + 511) // 512
            for s in range(NSEG):
                j0 = s * 512
                j1 = min(NQ, j0 + 512)
                pd = ps2.tile([COUT, 512], f32, space="PSUM", tag="den")
                nc.tensor.matmul(
                    out=pd[:, 0:j1 - j0],
                    lhsT=t_dw[:],
                    rhs=t_ftT[:, j0:j1],
                    start=True, stop=True)
                db = t_db[:, 0:1]
                odt = outp.tile([COUT, 512], f32, tag="odst")
                TT(out=odt[:, 0:j1 - j0], in0=pd[:, 0:j1 - j0],
                   in1=AP(db.tensor, db.offset, [db.ap[0], [0, j1 - j0]]),
                   op=ALU.add)
                nc.sync.dma_start(outdenseT[:, j0:j1], odt[:, 0:j1 - j0])

    nc.compile()
    return nc


# ----------------------------------------------------------------------------
# Entry point
# ----------------------------------------------------------------------------
def kernel(feats, pos, filt, dense_w, dense_b, src_idx, qry_idx):
    from concourse.bass_utils import run_bass_kernel_spmd

    feats = np.ascontiguousarray(np.asarray(feats, np.float32))
    pos = np.ascontiguousarray(np.asarray(pos, np.float32))
    filt = np.asarray(filt, np.float32)
    dense_w = np.asarray(dense_w, np.float32)
    dense_b = np.asarray(dense_b, np.float32)
    src_idx = np.asarray(src_idx).astype(np.int64)
    qry_idx = np.asarray(qry_idx).astype(np.int64)

    plans, bstart, bsz = _plan(qry_idx)
    NCH = max(len(p['chunks']) for p in plans)
    NCHP = ((NCH + 15) // 16) * 16
    NQ = NCHP * 16

    # filter regroup: G2[ax*32+c, t*64+o] = filt[az, ay, ax, c, o], t = az*4+ay
    G2 = np.zeros((128, 16 * 64), np.float32)
    for az in range(4):
        for ay in range(4):
            t = az * 4 + ay
            for ax in range(4):
                G2[ax * 32:(ax + 1) * 32, t * 64:(t + 1) * 64] = filt[az, ay, ax]

    in_maps = []
    for c, p in enumerate(plans):
        possrc, posqry, fsrc, qlocf = _pack_core(p, bstart, pos, feats,
                                                 qry_idx, src_idx, NCHP)
        ftT = np.zeros((CIN, NQ), np.float32)
        ftT[:, 0:p['nq']] = feats[p['q0']:p['q0'] + p['nq']].T
        in_maps.append({
            "possrc": possrc, "posqry": posqry, "fsrc": fsrc, "qlocf": qlocf,
            "g2": G2, "featsT": ftT, "denw": dense_w,
            "denb": dense_b.reshape(COUT, 1).astype(np.float32),
        })

    key = (NCHP, NQ)
    if key not in _COMPILED:
        _COMPILED[key] = _build_bass(NCHP, NQ)
    nc = _COMPILED[key]

    res = run_bass_kernel_spmd(nc, in_maps, core_ids=list(range(NCORES)))

    ans_conv = np.zeros((N, COUT), np.float32)
    ans_dense = np.zeros((N, COUT), np.float32)
    for c, p in enumerate(plans):
        outT = res.results[c]["outconvT"]
        for ci, (bA, bB) in enumerate(p['chunks']):
            for half, b in enumerate((bA, bB)):
                if b is None:
                    continue
                cols = ci * 16 + half * 8
                ans_conv[8 * b:8 * b + 8] = outT[:, cols:cols + 8].T
        dT = res.results[c]["outdenseT"]
        ans_dense[p['q0']:p['q0'] + p['nq']] = dT[:, 0:p['nq']].T
    return ans_conv, ans_dense



# revision 11
# speedup vs baseline: 1.4013x; 1.4013x over previous
"""Trainium2 Bass kernel for nn_ContinuousConvolutionBlock (gnn_message_passing).

Strategy (per sharding hint: partition points across 8 cores; each core owns its
queries' scatter-reduce and tap-GEMM; filter + dense weights replicated):

Host side (index plumbing / input marshalling only — zero FLOPs):
  - qry_idx is sorted; queries are grouped into 8-query blocks, blocks paired
    into 128-edge-slot "chunks" (two-pointer bin packing, ~3% padding).
  - Consecutive block ranges are assigned to the 8 cores; per-core per-slot
    payload arrays (pos[src], pos[qry] in coord-major layout, feats[src] in
    bf16, local query id) are marshalled on host and DMA'd densely.

Device side (all FLOP-bearing compute):
  - Geometry: ball->cube volume-preserving map on UNSCALED deltas (the map is
    linear in scale; the 2/EXTENT factor folds into the final grid transform),
    batched over all chunks in wide [128, 3W]-style ops; the 4-wide trilinear
    corner weights come from a hat-function identity w4[j] = relu(1 - |g - j|)
    (g in [0,3] for align_corners KS=4), computed in 2 DVE + 1 ACT ops.
  - Scatter-reduce as factored matmul per 128-slot chunk (bf16 operands,
    fp32 PSUM accumulate): R[slot,(ax,c)] = w4x (x) feats and
    L[slot,(half,q,t)] = Qoh (x) (w4z (x) w4y); PE computes
    A^T[(ax,c),(q,az,ay)] = R^T @ L. R/L for 16 chunks are built in 1-2
    wide ops (R on the Pool engine, L on DVE) instead of per-chunk ops.
  - Tap-GEMM: out^T += G_t^T @ A^T-slices over 16 (az,ay) taps, bf16.
  - Dense branch: out_dense^T = dense_w^T @ feats^T + b, bf16 matmul with
    per-partition bias add on ACT.
  Outputs are produced transposed ([64, nq]); host transposes/reorders back.
"""
import sys
import os
sys.path.insert(0, '/opt/trn_rl_repo')
import numpy as np
import ml_dtypes

BF16 = ml_dtypes.bfloat16

N = 30000
CIN = 32
COUT = 64
KS = 4
EXTENT = 0.08
NCORES = 8
NBLK = N // 8  # 3750 eight-query blocks

_COMPILED = {}


# ----------------------------------------------------------------------------
# Host planning
# ----------------------------------------------------------------------------
def _plan(qry_idx):
    deg = np.bincount(qry_idx, minlength=N)
    bsz = deg.reshape(NBLK, 8).sum(1)
    bstart = np.concatenate([[0], np.cumsum(bsz)]).astype(np.int64)
    per = [NBLK // NCORES + (1 if c < NBLK % NCORES else 0) for c in range(NCORES)]
    b0 = np.concatenate([[0], np.cumsum(per)]).astype(np.int64)
    plans = []
    for c in range(NCORES):
        blocks = list(range(b0[c], b0[c + 1]))
        asc = sorted(blocks, key=lambda b: bsz[b])
        chunks = []
        lo, hi = 0, len(asc) - 1
        while lo <= hi:
            if lo == hi:
                chunks.append((asc[hi], None)); break
            if bsz[asc[hi]] + bsz[asc[lo]] <= 128:
                chunks.append((asc[hi], asc[lo])); hi -= 1; lo += 1
            else:
                chunks.append((asc[hi], None)); hi -= 1
        plans.append(dict(blocks=blocks, chunks=chunks, q0=int(8 * b0[c]),
                          nq=int(8 * (b0[c + 1] - b0[c]))))
    return plans, bstart, bsz


def _pack_core(plan_c, bstart, pos, feats_bf, qry_idx, src_idx, NCHP):
    """Build per-slot payload arrays: pos coord-major, feats bf16."""
    possrc = np.zeros((128, 4, NCHP), np.float32)
    posqry = np.zeros((128, 4, NCHP), np.float32)
    fsrc = np.zeros((128, NCHP, CIN), BF16)
    qlocf = np.full((128, NCHP), -1.0, np.float32)
    for ci, (bA, bB) in enumerate(plan_c['chunks']):
        s = 0
        for half, b in enumerate((bA, bB)):
            if b is None:
                continue
            e0, e1 = int(bstart[b]), int(bstart[b + 1])
            n = e1 - e0
            sl = slice(s, s + n)
            possrc[sl, 0:3, ci] = pos[src_idx[e0:e1]]
            posqry[sl, 0:3, ci] = pos[qry_idx[e0:e1]]
            fsrc[sl, ci, :] = feats_bf[src_idx[e0:e1]]
            qlocf[sl, ci] = (qry_idx[e0:e1] - 8 * b) + 8 * half
            s += n
    return possrc, posqry, fsrc, qlocf


# ----------------------------------------------------------------------------
# Device kernel
# ----------------------------------------------------------------------------
def _build_bass(NCHP, NQ):
    import concourse.bass as bass
    import concourse.tile as tile
    from concourse import bacc, mybir
    from concourse.bass import AP

    f32 = mybir.dt.float32
    bf16 = mybir.dt.bfloat16
    i32 = mybir.dt.int32
    ALU = mybir.AluOpType
    ACT = mybir.ActivationFunctionType
    EPS = 1e-12
    F4PI = float(4.0 / np.pi)
    SC = 1.5 * (2.0 / EXTENT)  # grid scale folded with coord normalization

    nc = bacc.Bacc("TRN2", target_bir_lowering=False, debug=False)

    W = NCHP
    NGRP = W // 16

    possrc = nc.dram_tensor("possrc", (128, 4, W), f32, kind="ExternalInput")
    posqry = nc.dram_tensor("posqry", (128, 4, W), f32, kind="ExternalInput")
    fsrc = nc.dram_tensor("fsrc", (128, W, CIN), bf16, kind="ExternalInput")
    qlocf = nc.dram_tensor("qlocf", (128, W), f32, kind="ExternalInput")
    g2 = nc.dram_tensor("g2", (128, 16 * 64), bf16, kind="ExternalInput")
    featsT = nc.dram_tensor("featsT", (CIN, NQ), bf16, kind="ExternalInput")
    denw = nc.dram_tensor("denw", (CIN, COUT), bf16, kind="ExternalInput")
    denb = nc.dram_tensor("denb", (COUT, 1), f32, kind="ExternalInput")

    outconvT = nc.dram_tensor("outconvT", (COUT, NQ), f32, kind="ExternalOutput")
    outdenseT = nc.dram_tensor("outdenseT", (COUT, NQ), f32, kind="ExternalOutput")

    with tile.TileContext(nc) as tc:
        with tc.tile_pool(name="inp", bufs=1) as inp, \
             tc.tile_pool(name="geo", bufs=1) as geo, \
             tc.tile_pool(name="tmp", bufs=1) as tmp, \
             tc.tile_pool(name="lp", bufs=3) as lpool, \
             tc.tile_pool(name="rp", bufs=3) as rpool, \
             tc.tile_pool(name="at", bufs=3) as atp, \
             tc.tile_pool(name="outp", bufs=4) as outp, \
             tc.tile_pool(name="ps1", bufs=2, space="PSUM") as ps1, \
             tc.tile_pool(name="ps2", bufs=2, space="PSUM") as ps2:

            # ---------------- input DMAs ----------------
            t_ps = inp.tile([128, 4, W], f32)
            t_pq = inp.tile([128, 4, W], f32)
            t_f = inp.tile([128, W, CIN], bf16)
            t_ql = inp.tile([128, W], f32)
            t_g2 = inp.tile([128, 16 * 64], bf16)
            t_ftT = inp.tile([CIN, NQ], bf16)
            t_dw = inp.tile([CIN, COUT], bf16)
            t_db = inp.tile([COUT, 1], f32)
            nc.sync.dma_start(t_ps[:], possrc[:])
            nc.sync.dma_start(t_pq[:], posqry[:])
            nc.sync.dma_start(t_f[:], fsrc[:])
            nc.sync.dma_start(t_ql[:], qlocf[:])
            nc.sync.dma_start(t_g2[:], g2[:])
            nc.sync.dma_start(t_ftT[:], featsT[:])
            nc.sync.dma_start(t_dw[:], denw[:])
            nc.sync.dma_start(t_db[:], denb[:])

            # iota constants: io4m = j - 1.5 (j=0..3), io16 = 0..15
            io4i = tmp.tile([128, 4], i32)
            nc.gpsimd.iota(io4i[:], pattern=[[1, 4]], base=0, channel_multiplier=0)
            io4m = geo.tile([128, 4], f32)
            nc.scalar.activation(io4m[:], io4i[:], ACT.Copy, bias=-1.5)
            io16i = tmp.tile([128, 16], i32)
            nc.gpsimd.iota(io16i[:], pattern=[[1, 16]], base=0, channel_multiplier=0)
            io16 = geo.tile([128, 16], f32)
            nc.scalar.activation(io16[:], io16i[:], ACT.Copy)

            TT = nc.vector.tensor_tensor
            TS = nc.vector.tensor_scalar
            STT = nc.vector.scalar_tensor_tensor
            AA = nc.scalar.activation

            def fl(t, n):  # flat [128, n] view of a tile's first n elements
                return AP(t.tensor, t[:].offset, [t[:].ap[0], [1, n]])

            def sl(t, off, n, *dims):  # strided view: dims = (stride, count)*
                pat = [t[:].ap[0]] + [[s, c] for (s, c) in dims] if dims else \
                      [t[:].ap[0], [1, n]]
                return AP(t.tensor, t[:].offset + off, pat)

            # ---------------- geometry ----------------
            # temp tiles (tag-free; geo pool is one-shot so SBUF is reused
            # only via distinct names — sized modestly)
            dd = geo.tile([128, 3, W], f32)
            sq3 = geo.tile([128, 3, W], f32)
            TT(out=fl(dd, 3 * W), in0=fl(t_ps, 3 * W), in1=fl(t_pq, 3 * W),
               op=ALU.subtract)
            TT(out=fl(sq3, 3 * W), in0=fl(dd, 3 * W), in1=fl(dd, 3 * W),
               op=ALU.mult)

            def gW(name):
                return geo.tile([128, W], f32, name=name)

            xy2 = gW("xy2"); sq = gW("sq"); norm = gW("norm"); nxy = gW("nxy")
            azn = gW("azn"); den1 = gW("den1"); rd1 = gW("rd1"); t1s = gW("t1s")
            s1 = gW("s1"); rd2 = gW("rd2"); s2 = gW("s2")
            pole = geo.tile([128, W], i32, name="pole")
            wq = gW("wq"); zsg = gW("zsg"); zcp = gW("zcp")
            sqxy = gW("sqxy"); nrm = gW("nrm")
            abr = geo.tile([128, W], i32, name="abr")

            zofs = 2 * W  # z slice offset in dd/sq3
            TT(out=xy2[:], in0=sl(sq3, 0, W), in1=sl(sq3, W, W), op=ALU.add)
            TT(out=sq[:], in0=xy2[:], in1=sl(sq3, zofs, W), op=ALU.add)
            AA(norm[:], sq[:], ACT.Sqrt)
            AA(nxy[:], xy2[:], ACT.Sqrt)
            AA(azn[:], sl(dd, zofs, W), ACT.Abs)
            STT(out=den1[:], in0=azn[:], scalar=EPS, in1=norm[:],
                op0=ALU.add, op1=ALU.add)
            nc.vector.reciprocal_approx_fast(rd1[:], den1[:])
            TT(out=t1s[:], in0=norm[:], in1=rd1[:], op=ALU.mult)
            AA(s1[:], t1s[:], ACT.Sqrt, scale=3.0)
            den2 = gW("den2")
            TS(den2[:], nxy[:], EPS, None, op0=ALU.add)
            nc.vector.reciprocal_approx_fast(rd2[:], den2[:])
            TT(out=s2[:], in0=norm[:], in1=rd2[:], op=ALU.mult)
            STT(out=pole[:], in0=sl(sq3, zofs, W), scalar=1.25, in1=xy2[:],
                op0=ALU.mult, op1=ALU.is_gt)
            # wq = where(pole, s1, s2)
            nc.vector.tensor_copy(wq[:], s2[:])
            nc.vector.copy_predicated(wq[:], pole[:], s1[:])

            m3 = geo.tile([128, 3, W], f32)
            # zc = where(pole, sign(z)*norm, 1.5 z)
            AA(zsg[:], sl(dd, zofs, W), ACT.Sign)
            TT(out=zcp[:], in0=zsg[:], in1=norm[:], op=ALU.mult)
            TS(sl(m3, zofs, W), sl(dd, zofs, W), 1.5, None, op0=ALU.mult)
            nc.vector.copy_predicated(sl(m3, zofs, W), pole[:], zcp[:])

            # xc, yc = (x, y) * wq   [128, 2, W]
            xyc = geo.tile([128, 2, W], f32)
            TT(out=sl(xyc, 0, 0, (W, 2), (1, W)),
               in0=sl(dd, 0, 0, (W, 2), (1, W)),
               in1=sl(wq, 0, 0, (0, 2), (1, W)), op=ALU.mult)
            xyc2 = geo.tile([128, 2, W], f32)
            TT(out=fl(xyc2, 2 * W), in0=fl(xyc, 2 * W), in1=fl(xyc, 2 * W),
               op=ALU.mult)
            TT(out=sqxy[:], in0=sl(xyc2, 0, W), in1=sl(xyc2, W, W), op=ALU.add)
            AA(nrm[:], sqxy[:], ACT.Sqrt)
            axy = geo.tile([128, 2, W], f32)
            AA(fl(axy, 2 * W), fl(xyc, 2 * W), ACT.Abs)
            TT(out=abr[:], in0=sl(axy, W, W), in1=sl(axy, 0, W), op=ALU.is_le)
            # safe denominators + reciprocals
            myx = geo.tile([128, 2, W], f32)
            TS(fl(myx, 2 * W), fl(axy, 2 * W), EPS, None, op0=ALU.is_lt)
            sf = geo.tile([128, 2, W], f32)
            TT(out=fl(sf, 2 * W), in0=fl(xyc, 2 * W), in1=fl(myx, 2 * W),
               op=ALU.add)
            rsf = geo.tile([128, 2, W], f32)
            nc.vector.reciprocal_approx_fast(fl(rsf, 2 * W), fl(sf, 2 * W))
            # cross ratios: ratx = xc/sfy, raty = yc/sfx
            rat = geo.tile([128, 2, W], f32)
            TT(out=sl(rat, 0, W), in0=sl(xyc, 0, W), in1=sl(rsf, W, W),
               op=ALU.mult)
            TT(out=sl(rat, W, W), in0=sl(xyc, W, W), in1=sl(rsf, 0, W),
               op=ALU.mult)
            at12 = geo.tile([128, 2, W], f32)
            AA(fl(at12, 2 * W), fl(rat, 2 * W), ACT.Arctan)
            sg = geo.tile([128, 2, W], f32)
            AA(fl(sg, 2 * W), fl(xyc, 2 * W), ACT.Sign)
            tmpab = geo.tile([128, 2, W], f32)
            TT(out=sl(tmpab, 0, 0, (W, 2), (1, W)),
               in0=sl(sg, 0, 0, (W, 2), (1, W)),
               in1=sl(nrm, 0, 0, (0, 2), (1, W)), op=ALU.mult)
            # xoe = tmpb * F4PI * at1 ; yoe = tmpa * F4PI * at2
            prod = geo.tile([128, 2, W], f32)
            STT(out=sl(prod, 0, W), in0=sl(at12, 0, W), scalar=F4PI,
                in1=sl(tmpab, W, W), op0=ALU.mult, op1=ALU.mult)
            STT(out=sl(prod, W, W), in0=sl(at12, W, W), scalar=F4PI,
                in1=sl(tmpab, 0, W), op0=ALU.mult, op1=ALU.mult)
            # m3x = where(abr, tmpa, xoe); m3y = where(abr, yoe, tmpb)
            nc.vector.tensor_copy(sl(m3, 0, W), sl(prod, 0, W))
            nc.vector.copy_predicated(sl(m3, 0, W), abr[:], sl(tmpab, 0, W))
            nc.vector.tensor_copy(sl(m3, W, W), sl(tmpab, W, W))
            nc.vector.copy_predicated(sl(m3, W, W), abr[:], sl(prod, W, W))

            # hat-function corner weights: w4[j] = relu(1 - |SC*m - (j-1.5)|)
            d4 = geo.tile([128, 3 * W, 4], f32)
            STT(out=sl(d4, 0, 0, (4, 3 * W), (1, 4)),
                in0=sl(m3, 0, 0, (1, 3 * W), (0, 4)),
                scalar=SC,
                in1=sl(io4m, 0, 0, (0, 3 * W), (1, 4)),
                op0=ALU.mult, op1=ALU.subtract)
            nd4 = geo.tile([128, 3 * W, 4], f32)
            AA(fl(nd4, 12 * W), fl(d4, 12 * W), ACT.Abs)
            w4b = geo.tile([128, 3, W, 4], bf16)
            AA(fl(w4b, 12 * W), fl(nd4, 12 * W), ACT.Relu, bias=1.0, scale=-1.0)

            # qoh [128, W, 16] and zy [128, W, 16] (bf16)
            qoh = geo.tile([128, W, 16], bf16)
            TT(out=qoh[:],
               in0=sl(t_ql, 0, 0, (1, W), (0, 16)),
               in1=sl(io16, 0, 0, (0, W), (1, 16)),
               op=ALU.is_equal)
            zy = geo.tile([128, W, 16], bf16)
            TT(out=zy[:],
               in0=sl(w4b, 2 * W * 4, 0, (4, W), (1, 4), (0, 4)),
               in1=sl(w4b, 1 * W * 4, 0, (4, W), (0, 4), (1, 4)),
               op=ALU.mult)

            # ---------------- stage-1 + tap-GEMM ----------------
            for g in range(NGRP):
                ci0 = g * 16
                # R for 16 chunks in one Pool op: [128, 16, 128] bf16
                R_t = rpool.tile([128, 16, 128], bf16, tag="R")
                nc.gpsimd.tensor_tensor(
                    out=sl(R_t, 0, 0, (128, 16), (32, 4), (1, 32)),
                    in0=sl(w4b, ci0 * 4, 0, (4, 16), (1, 4), (0, 32)),
                    in1=sl(t_f, ci0 * CIN, 0, (32, 16), (0, 4), (1, 32)),
                    op=ALU.mult)
                # L for 16 chunks in two DVE ops: [128, 16, 256] bf16
                L_t = lpool.tile([128, 16, 256], bf16, tag="L")
                for h in range(2):
                    TT(out=sl(L_t, h * 128, 0, (256, 16), (16, 8), (1, 16)),
                       in0=sl(qoh, ci0 * 16 + h * 8, 0, (16, 16), (1, 8), (0, 16)),
                       in1=sl(zy, ci0 * 16, 0, (16, 16), (0, 8), (1, 16)),
                       op=ALU.mult)
                at_t = atp.tile([128, 16 * 256], bf16, tag="at")
                for j in range(4):
                    ps_t = ps1.tile([128, 1024], f32, space="PSUM", tag="s1")
                    for r in range(4):
                        k = j * 4 + r
                        nc.tensor.matmul(
                            out=ps_t[:, r * 256:(r + 1) * 256],
                            lhsT=R_t[:, k, :], rhs=L_t[:, k, :],
                            start=True, stop=True)
                    dst = at_t[:, j * 1024:(j + 1) * 1024]
                    if j == 3:
                        nc.vector.tensor_copy(dst, ps_t[:])
                    else:
                        nc.scalar.copy(dst, ps_t[:])
                # tap-GEMM for this group
                po = ps2.tile([COUT, 256], f32, space="PSUM", tag="tap")
                for t in range(16):
                    rhs = AP(at_t.tensor, at_t[:].offset + t,
                             [at_t[:].ap[0], [256, 16], [128, 2], [16, 8]])
                    nc.tensor.matmul(
                        out=po[:],
                        lhsT=t_g2[:, t * 64:(t + 1) * 64],
                        rhs=rhs,
                        start=(t == 0), stop=(t == 15))
                ost = outp.tile([COUT, 256], f32, tag="ocst")
                nc.vector.tensor_copy(ost[:], po[:])
                nc.sync.dma_start(outconvT[:, g * 256:(g + 1) * 256], ost[:])

            # ---------------- dense branch (bf16 matmul + ACT bias) --------
            NSEG = (NQ + 511) // 512
            for s in range(NSEG):
                j0 = s * 512
                j1 = min(NQ, j0 + 512)
                pd = ps2.tile([COUT, 512], f32, space="PSUM", tag="den")
                nc.tensor.matmul(
                    out=pd[:, 0:j1 - j0],
                    lhsT=t_dw[:],
                    rhs=t_ftT[:, j0:j1],
                    start=True, stop=True)
                odt = outp.tile([COUT, 512], f32, tag="odst")
                nc.scalar.activation(odt[:, 0:j1 - j0], pd[:, 0:j1 - j0],
                                     ACT.Identity, bias=t_db[:, 0:1])
                nc.sync.dma_start(outdenseT[:, j0:j1], odt[:, 0:j1 - j0])

    nc.compile()
    return nc


# ----------------------------------------------------------------------------
# Entry point
# ----------------------------------------------------------------------------
def _prepare(feats, pos, filt, dense_w, dense_b, src_idx, qry_idx):
    feats = np.ascontiguousarray(np.asarray(feats, np.float32))
    pos = np.ascontiguousarray(np.asarray(pos, np.float32))
    filt = np.asarray(filt, np.float32)
    dense_w = np.asarray(dense_w, np.float32)
    dense_b = np.asarray(dense_b, np.float32)
    src_idx = np.asarray(src_idx).astype(np.int64)
    qry_idx = np.asarray(qry_idx).astype(np.int64)

    plans, bstart, bsz = _plan(qry_idx)
    NCH = max(len(p['chunks']) for p in plans)
    NCHP = ((NCH + 15) // 16) * 16
    NQ = NCHP * 16

    feats_bf = feats.astype(BF16)

    # filter regroup: G2[ax*32+c, t*64+o] = filt[az, ay, ax, c, o], t = az*4+ay
    G2 = np.zeros((128, 16 * 64), np.float32)
    for az in range(4):
        for ay in range(4):
            t = az * 4 + ay
            for ax in range(4):
                G2[ax * 32:(ax + 1) * 32, t * 64:(t + 1) * 64] = filt[az, ay, ax]
    G2b = G2.astype(BF16)

    in_maps = []
    for c, p in enumerate(plans):
        possrc, posqry, fsrc, qlocf = _pack_core(p, bstart, pos, feats_bf,
                                                 qry_idx, src_idx, NCHP)
        ftT = np.zeros((CIN, NQ), BF16)
        ftT[:, 0:p['nq']] = feats_bf[p['q0']:p['q0'] + p['nq']].T
        in_maps.append({
            "possrc": possrc, "posqry": posqry, "fsrc": fsrc, "qlocf": qlocf,
            "g2": G2b, "featsT": ftT, "denw": dense_w.astype(BF16),
            "denb": dense_b.reshape(COUT, 1).astype(np.float32),
        })

    key = (NCHP, NQ)
    if key not in _COMPILED:
        _COMPILED[key] = _build_bass(NCHP, NQ)
    nc = _COMPILED[key]
    return nc, in_maps, plans


def kernel(feats, pos, filt, dense_w, dense_b, src_idx, qry_idx):
    from concourse.bass_utils import run_bass_kernel_spmd

    nc, in_maps, plans = _prepare(feats, pos, filt, dense_w, dense_b,
                                  src_idx, qry_idx)
    res = run_bass_kernel_spmd(nc, in_maps, core_ids=list(range(NCORES)))

    ans_conv = np.zeros((N, COUT), np.float32)
    ans_dense = np.zeros((N, COUT), np.float32)
    for c, p in enumerate(plans):
        outT = res.results[c]["outconvT"]
        for ci, (bA, bB) in enumerate(p['chunks']):
            for half, b in enumerate((bA, bB)):
                if b is None:
                    continue
                cols = ci * 16 + half * 8
                ans_conv[8 * b:8 * b + 8] = outT[:, cols:cols + 8].T
        dT = res.results[c]["outdenseT"]
        ans_dense[p['q0']:p['q0'] + p['nq']] = dT[:, 0:p['nq']].T
    return ans_conv, ans_dense


# revision 18
# speedup vs baseline: 2.2905x; 1.6346x over previous
"""Trainium2 Bass kernel for nn_ContinuousConvolutionBlock (gnn_message_passing).

Strategy (per sharding hint: partition points across 8 cores; each core owns its
queries' scatter-reduce and tap-GEMM; filter + dense weights replicated):

Host side (index plumbing / input marshalling only — zero FLOPs):
  - qry_idx is sorted; queries are grouped into 8-query blocks, blocks paired
    into 128-edge-slot "chunks" (two-pointer bin packing, ~3% padding).
  - Consecutive block ranges are assigned to the 8 cores; per-core per-slot
    payload arrays (pos[src]/pos[qry] coord-major, feats[src] bf16, int16
    scatter indices qloc*16+t) are marshalled on host and DMA'd densely.

Device side (all FLOP-bearing compute), work spread over all five engines:
  - Geometry (DVE+ACT): ball->cube volume-preserving map on UNSCALED deltas
    (map is linear in scale; 2/EXTENT folds into the final grid transform),
    batched over all chunks in wide [128, 3W]-style ops; 4-wide trilinear
    corner weights via the hat identity w4[j] = relu(1 - |g - j|).
  - L build (Pool): L[slot, (q16, az, ay)] = one-hot scatter of
    zy = w4z (x) w4y into the query slot — gpsimd local_scatter with
    host-precomputed per-partition indices (zeroes dst, skips -1 padding).
  - R build (DVE): R[slot, (ax,c)] = w4x (x) feats, bf16, 16 chunks/op.
  - Stage-1 (PE): A^T[(ax,c), (q,az,ay)] = R^T @ L per chunk, bf16 operands,
    fp32 PSUM accumulate.
  - PSUM -> SBUF staging via the DMA engine (f32, no cast), freeing DVE/ACT.
  - Tap-GEMM (PE): out^T += G_t^T @ A^T-slices over 16 taps in f32r
    (1 cy/row at 256-row moving dim); output DMA'd straight from PSUM.
  - Dense branch (PE, bf16) issued first so PE warms during geometry.
  Tap-GEMM of group g-1 is interleaved with stage-1 of group g to keep the
  PE stream continuous.
"""
import sys
import os
sys.path.insert(0, '/opt/trn_rl_repo')
import numpy as np
import ml_dtypes

BF16 = ml_dtypes.bfloat16

N = 30000
CIN = 32
COUT = 64
KS = 4
EXTENT = 0.08
NCORES = 8
NBLK = N // 8  # 3750 eight-query blocks

_COMPILED = {}


# ----------------------------------------------------------------------------
# Host planning
# ----------------------------------------------------------------------------
def _plan(qry_idx):
    deg = np.bincount(qry_idx, minlength=N)
    bsz = deg.reshape(NBLK, 8).sum(1)
    bstart = np.concatenate([[0], np.cumsum(bsz)]).astype(np.int64)
    per = [NBLK // NCORES + (1 if c < NBLK % NCORES else 0) for c in range(NCORES)]
    b0 = np.concatenate([[0], np.cumsum(per)]).astype(np.int64)
    plans = []
    for c in range(NCORES):
        blocks = list(range(b0[c], b0[c + 1]))
        asc = sorted(blocks, key=lambda b: bsz[b])
        chunks = []
        lo, hi = 0, len(asc) - 1
        while lo <= hi:
            if lo == hi:
                chunks.append((asc[hi], None)); break
            if bsz[asc[hi]] + bsz[asc[lo]] <= 128:
                chunks.append((asc[hi], asc[lo])); hi -= 1; lo += 1
            else:
                chunks.append((asc[hi], None)); hi -= 1
        plans.append(dict(blocks=blocks, chunks=chunks, q0=int(8 * b0[c]),
                          nq=int(8 * (b0[c + 1] - b0[c]))))
    return plans, bstart, bsz


def _pack_core(plan_c, bstart, pos, feats_bf, qry_idx, src_idx, NCHP):
    """Per-slot payload: pos coord-major, feats bf16, int16 scatter indices."""
    possrc = np.zeros((128, 4, NCHP), np.float32)
    posqry = np.zeros((128, 4, NCHP), np.float32)
    fsrc = np.zeros((128, NCHP, CIN), BF16)
    qloc = np.full((128, NCHP), -1, np.int32)
    for ci, (bA, bB) in enumerate(plan_c['chunks']):
        s = 0
        for half, b in enumerate((bA, bB)):
            if b is None:
                continue
            e0, e1 = int(bstart[b]), int(bstart[b + 1])
            n = e1 - e0
            sl = slice(s, s + n)
            possrc[sl, 0:3, ci] = pos[src_idx[e0:e1]]
            posqry[sl, 0:3, ci] = pos[qry_idx[e0:e1]]
            fsrc[sl, ci, :] = feats_bf[src_idx[e0:e1]]
            qloc[sl, ci] = (qry_idx[e0:e1] - 8 * b) + 8 * half
            s += n
    # scatter index: within each 4-chunk scatter window,
    # idx[slot, ci, t] = (ci%4)*256 + qloc*16 + t  (or -1 for padding)
    t16 = np.arange(16, dtype=np.int32)
    idx = ((qloc % 4 * 0) + (np.arange(NCHP, dtype=np.int32)[None, :] % 4) * 256
           + qloc * 16)[:, :, None] + t16[None, None, :]
    idx = np.where((qloc < 0)[:, :, None], -1, idx).astype(np.int16)
    return possrc, posqry, fsrc, idx


# ----------------------------------------------------------------------------
# Device kernel
# ----------------------------------------------------------------------------
def _build_bass(NCHP, NQ):
    import concourse.bass as bass
    import concourse.tile as tile
    from concourse import bacc, mybir
    from concourse.bass import AP

    f32 = mybir.dt.float32
    f32r = mybir.dt.float32r
    bf16 = mybir.dt.bfloat16
    i32 = mybir.dt.int32
    i16 = mybir.dt.int16
    ALU = mybir.AluOpType
    ACT = mybir.ActivationFunctionType
    EPS = 1e-12
    F4PI = float(4.0 / np.pi)
    SC = 1.5 * (2.0 / EXTENT)  # grid scale folded with coord normalization

    nc = bacc.Bacc("TRN2", target_bir_lowering=False, debug=False)

    W = NCHP
    NGRP = W // 16

    possrc = nc.dram_tensor("possrc", (128, 4, W), f32, kind="ExternalInput")
    posqry = nc.dram_tensor("posqry", (128, 4, W), f32, kind="ExternalInput")
    fsrc = nc.dram_tensor("fsrc", (128, W, CIN), bf16, kind="ExternalInput")
    sidx = nc.dram_tensor("sidx", (128, W, 16), i16, kind="ExternalInput")
    g2 = nc.dram_tensor("g2", (128, 16 * 64), f32, kind="ExternalInput")
    featsT = nc.dram_tensor("featsT", (CIN, NQ), bf16, kind="ExternalInput")
    denw = nc.dram_tensor("denw", (CIN, COUT), bf16, kind="ExternalInput")
    denb = nc.dram_tensor("denb", (COUT, 1), f32, kind="ExternalInput")

    outconvT = nc.dram_tensor("outconvT", (COUT, NQ), f32, kind="ExternalOutput")
    outdenseT = nc.dram_tensor("outdenseT", (COUT, NQ), f32, kind="ExternalOutput")

    with tile.TileContext(nc) as tc:
        with tc.tile_pool(name="inp", bufs=1) as inp, \
             tc.tile_pool(name="geo", bufs=1) as geo, \
             tc.tile_pool(name="tmp", bufs=1) as tmp, \
             tc.tile_pool(name="lp", bufs=3) as lpool, \
             tc.tile_pool(name="rp", bufs=3) as rpool, \
             tc.tile_pool(name="at", bufs=2) as atp, \
             tc.tile_pool(name="outp", bufs=2) as outp, \
             tc.tile_pool(name="ps1", bufs=2, space="PSUM") as ps1, \
             tc.tile_pool(name="ps2", bufs=2, space="PSUM") as ps2, \
             tc.tile_pool(name="ps3", bufs=2, space="PSUM") as ps3:

            # ---------------- input DMAs ----------------
            t_ps = inp.tile([128, 4, W], f32)
            t_pq = inp.tile([128, 4, W], f32)
            t_f = inp.tile([128, W, CIN], bf16)
            t_si = inp.tile([128, W, 16], i16)
            t_g2 = inp.tile([128, 16 * 64], f32)
            t_ftT = inp.tile([CIN, NQ], bf16)
            t_dw = inp.tile([CIN, COUT], bf16)
            t_db = inp.tile([COUT, 1], f32)
            nc.sync.dma_start(t_ps[:], possrc[:])
            nc.sync.dma_start(t_pq[:], posqry[:])
            nc.sync.dma_start(t_ftT[:], featsT[:])
            nc.sync.dma_start(t_dw[:], denw[:])
            nc.sync.dma_start(t_db[:], denb[:])
            nc.sync.dma_start(t_f[:], fsrc[:])
            nc.sync.dma_start(t_si[:], sidx[:])
            nc.sync.dma_start(t_g2[:], g2[:])

            # round filter to f32r once
            t_g2r = inp.tile([128, 16 * 64], f32r)
            nc.vector.tensor_copy(t_g2r[:], t_g2[:])

            # iota constants: io4m = j - 1.5 (j=0..3)
            io4i = tmp.tile([128, 4], i32)
            nc.gpsimd.iota(io4i[:], pattern=[[1, 4]], base=0, channel_multiplier=0)
            io4m = geo.tile([128, 4], f32)
            nc.scalar.activation(io4m[:], io4i[:], ACT.Copy, bias=-1.5)

            TT = nc.vector.tensor_tensor
            TS = nc.vector.tensor_scalar
            STT = nc.vector.scalar_tensor_tensor
            AA = nc.scalar.activation

            def fl(t, n):  # flat [128, n] view of a tile's first n elements
                return AP(t.tensor, t[:].offset, [t[:].ap[0], [1, n]])

            def sl(t, off, n, *dims):  # strided view: dims = (stride, count)*
                pat = [t[:].ap[0]] + [[s, c] for (s, c) in dims] if dims else \
                      [t[:].ap[0], [1, n]]
                return AP(t.tensor, t[:].offset + off, pat)

            # ------------- dense branch first (warms PE) -------------
            NSEG = (NQ + 511) // 512
            for s in range(NSEG):
                j0 = s * 512
                j1 = min(NQ, j0 + 512)
                pd = ps3.tile([COUT, 512], f32, space="PSUM", tag="den")
                nc.tensor.matmul(
                    out=pd[:, 0:j1 - j0],
                    lhsT=t_dw[:],
                    rhs=t_ftT[:, j0:j1],
                    start=True, stop=True)
                odt = outp.tile([COUT, 512], f32, tag="odst")
                nc.scalar.activation(odt[:, 0:j1 - j0], pd[:, 0:j1 - j0],
                                     ACT.Identity, bias=t_db[:, 0:1])
                nc.sync.dma_start(outdenseT[:, j0:j1], odt[:, 0:j1 - j0])

            # ---------------- geometry ----------------
            dd = geo.tile([128, 3, W], f32)
            sq3 = geo.tile([128, 3, W], f32)
            TT(out=fl(dd, 3 * W), in0=fl(t_ps, 3 * W), in1=fl(t_pq, 3 * W),
               op=ALU.subtract)
            TT(out=fl(sq3, 3 * W), in0=fl(dd, 3 * W), in1=fl(dd, 3 * W),
               op=ALU.mult)

            def gW(name):
                return geo.tile([128, W], f32, name=name)

            xy2 = gW("xy2"); sq = gW("sq"); norm = gW("norm"); nxy = gW("nxy")
            azn = gW("azn"); den1 = gW("den1"); rd1 = gW("rd1"); t1s = gW("t1s")
            s1 = gW("s1"); den2 = gW("den2"); rd2 = gW("rd2"); s2 = gW("s2")
            pole = geo.tile([128, W], i32, name="pole")
            wq = gW("wq"); zsg = gW("zsg"); zcp = gW("zcp")
            sqxy = gW("sqxy"); nrm = gW("nrm")
            abr = geo.tile([128, W], i32, name="abr")

            zofs = 2 * W  # z slice offset in dd/sq3
            TT(out=xy2[:], in0=sl(sq3, 0, W), in1=sl(sq3, W, W), op=ALU.add)
            TT(out=sq[:], in0=xy2[:], in1=sl(sq3, zofs, W), op=ALU.add)
            AA(norm[:], sq[:], ACT.Sqrt)
            AA(nxy[:], xy2[:], ACT.Sqrt)
            AA(azn[:], sl(dd, zofs, W), ACT.Abs)
            STT(out=den1[:], in0=azn[:], scalar=EPS, in1=norm[:],
                op0=ALU.add, op1=ALU.add)
            nc.vector.reciprocal_approx_fast(rd1[:], den1[:])
            TT(out=t1s[:], in0=norm[:], in1=rd1[:], op=ALU.mult)
            AA(s1[:], t1s[:], ACT.Sqrt, scale=3.0)
            TS(den2[:], nxy[:], EPS, None, op0=ALU.add)
            nc.vector.reciprocal_approx_fast(rd2[:], den2[:])
            TT(out=s2[:], in0=norm[:], in1=rd2[:], op=ALU.mult)
            STT(out=pole[:], in0=sl(sq3, zofs, W), scalar=1.25, in1=xy2[:],
                op0=ALU.mult, op1=ALU.is_gt)
            # wq = where(pole, s1, s2)
            nc.vector.tensor_copy(wq[:], s2[:])
            nc.vector.copy_predicated(wq[:], pole[:], s1[:])

            m3 = geo.tile([128, 3, W], f32)
            # zc = where(pole, sign(z)*norm, 1.5 z)
            AA(zsg[:], sl(dd, zofs, W), ACT.Sign)
            TT(out=zcp[:], in0=zsg[:], in1=norm[:], op=ALU.mult)
            TS(sl(m3, zofs, W), sl(dd, zofs, W), 1.5, None, op0=ALU.mult)
            nc.vector.copy_predicated(sl(m3, zofs, W), pole[:], zcp[:])

            # xc, yc = (x, y) * wq   [128, 2, W]
            xyc = geo.tile([128, 2, W], f32)
            TT(out=sl(xyc, 0, 0, (W, 2), (1, W)),
               in0=sl(dd, 0, 0, (W, 2), (1, W)),
               in1=sl(wq, 0, 0, (0, 2), (1, W)), op=ALU.mult)
            xyc2 = geo.tile([128, 2, W], f32)
            TT(out=fl(xyc2, 2 * W), in0=fl(xyc, 2 * W), in1=fl(xyc, 2 * W),
               op=ALU.mult)
            TT(out=sqxy[:], in0=sl(xyc2, 0, W), in1=sl(xyc2, W, W), op=ALU.add)
            AA(nrm[:], sqxy[:], ACT.Sqrt)
            axy = geo.tile([128, 2, W], f32)
            AA(fl(axy, 2 * W), fl(xyc, 2 * W), ACT.Abs)
            TT(out=abr[:], in0=sl(axy, W, W), in1=sl(axy, 0, W), op=ALU.is_le)
            # safe denominators + reciprocals
            myx = geo.tile([128, 2, W], f32)
            TS(fl(myx, 2 * W), fl(axy, 2 * W), EPS, None, op0=ALU.is_lt)
            sf = geo.tile([128, 2, W], f32)
            TT(out=fl(sf, 2 * W), in0=fl(xyc, 2 * W), in1=fl(myx, 2 * W),
               op=ALU.add)
            rsf = geo.tile([128, 2, W], f32)
            nc.vector.reciprocal_approx_fast(fl(rsf, 2 * W), fl(sf, 2 * W))
            # cross ratios: ratx = xc/sfy, raty = yc/sfx
            rat = geo.tile([128, 2, W], f32)
            TT(out=sl(rat, 0, W), in0=sl(xyc, 0, W), in1=sl(rsf, W, W),
               op=ALU.mult)
            TT(out=sl(rat, W, W), in0=sl(xyc, W, W), in1=sl(rsf, 0, W),
               op=ALU.mult)
            at12 = geo.tile([128, 2, W], f32)
            AA(fl(at12, 2 * W), fl(rat, 2 * W), ACT.Arctan)
            sg = geo.tile([128, 2, W], f32)
            AA(fl(sg, 2 * W), fl(xyc, 2 * W), ACT.Sign)
            tmpab = geo.tile([128, 2, W], f32)
            TT(out=sl(tmpab, 0, 0, (W, 2), (1, W)),
               in0=sl(sg, 0, 0, (W, 2), (1, W)),
               in1=sl(nrm, 0, 0, (0, 2), (1, W)), op=ALU.mult)
            # xoe = tmpb * F4PI * at1 ; yoe = tmpa * F4PI * at2
            prod = geo.tile([128, 2, W], f32)
            STT(out=sl(prod, 0, W), in0=sl(at12, 0, W), scalar=F4PI,
                in1=sl(tmpab, W, W), op0=ALU.mult, op1=ALU.mult)
            STT(out=sl(prod, W, W), in0=sl(at12, W, W), scalar=F4PI,
                in1=sl(tmpab, 0, W), op0=ALU.mult, op1=ALU.mult)
            # m3x = where(abr, tmpa, xoe); m3y = where(abr, yoe, tmpb)
            nc.vector.tensor_copy(sl(m3, 0, W), sl(prod, 0, W))
            nc.vector.copy_predicated(sl(m3, 0, W), abr[:], sl(tmpab, 0, W))
            nc.vector.tensor_copy(sl(m3, W, W), sl(tmpab, W, W))
            nc.vector.copy_predicated(sl(m3, W, W), abr[:], sl(prod, W, W))

            # hat-function corner weights: w4[j] = relu(1 - |SC*m - (j-1.5)|)
            d4 = geo.tile([128, 3 * W, 4], f32)
            STT(out=sl(d4, 0, 0, (4, 3 * W), (1, 4)),
                in0=sl(m3, 0, 0, (1, 3 * W), (0, 4)),
                scalar=SC,
                in1=sl(io4m, 0, 0, (0, 3 * W), (1, 4)),
                op0=ALU.mult, op1=ALU.subtract)
            nd4 = geo.tile([128, 3 * W, 4], f32)
            AA(fl(nd4, 12 * W), fl(d4, 12 * W), ACT.Abs)
            w4b = geo.tile([128, 3, W, 4], bf16)
            AA(fl(w4b, 12 * W), fl(nd4, 12 * W), ACT.Relu, bias=1.0, scale=-1.0)

            # zy [128, W, 16] bf16 (scatter payload)
            zy = geo.tile([128, W, 16], bf16)
            TT(out=zy[:],
               in0=sl(w4b, 2 * W * 4, 0, (4, W), (1, 4), (0, 4)),
               in1=sl(w4b, 1 * W * 4, 0, (4, W), (0, 4), (1, 4)),
               op=ALU.mult)

            # ---------------- stage-1 + tap-GEMM (pipelined) -------------
            pend = None  # deferred tap-GEMM work: (at_t, g)

            def tap_gemm(at_t, g):
                po = ps2.tile([COUT, 256], f32, space="PSUM", tag="tap")
                for t in range(16):
                    rhs = AP(at_t.tensor, at_t[:].offset + t,
                             [at_t[:].ap[0], [256, 16], [128, 2], [16, 8]])
                    nc.tensor.matmul(
                        out=po[:],
                        lhsT=t_g2r[:, t * 64:(t + 1) * 64],
                        rhs=rhs,
                        start=(t == 0), stop=(t == 15))
                ost = outp.tile([COUT, 256], f32, tag="ocst")
                nc.scalar.copy(ost[:], po[:])
                nc.sync.dma_start(outconvT[:, g * 256:(g + 1) * 256], ost[:])

            for g in range(NGRP):
                ci0 = g * 16
                # R for 16 chunks in one DVE op: [128, 16, 128] bf16
                R_t = rpool.tile([128, 16, 128], bf16, tag="R")
                TT(out=sl(R_t, 0, 0, (128, 16), (32, 4), (1, 32)),
                   in0=sl(w4b, ci0 * 4, 0, (4, 16), (1, 4), (0, 32)),
                   in1=sl(t_f, ci0 * CIN, 0, (32, 16), (0, 4), (1, 32)),
                   op=ALU.mult)
                # L via gpsimd local_scatter, 4 chunks per op
                L_t = lpool.tile([128, 16, 256], bf16, tag="L")
                for j in range(4):
                    c0 = ci0 + 4 * j
                    nc.gpsimd.local_scatter(
                        out_ap=sl(L_t, j * 1024, 1024),
                        data_ap=sl(zy, c0 * 16, 64),
                        idxs_ap=sl(t_si, c0 * 16, 64),
                        channels=128, num_elems=1024, num_idxs=64)
                at_t = atp.tile([128, 16 * 256], f32r, tag="at")
                for j in range(4):
                    ps_t = ps1.tile([128, 1024], f32, space="PSUM", tag="s1")
                    for r in range(4):
                        k = j * 4 + r
                        nc.tensor.matmul(
                            out=ps_t[:, r * 256:(r + 1) * 256],
                            lhsT=R_t[:, k, :], rhs=L_t[:, k, :],
                            start=True, stop=True)
                    dst = at_t[:, j * 1024:(j + 1) * 1024]
                    if j == 3:
                        nc.vector.tensor_copy(dst, ps_t[:])
                    else:
                        nc.scalar.copy(dst, ps_t[:])
                if pend is not None:
                    tap_gemm(*pend)
                pend = (at_t, g)
            tap_gemm(*pend)

    nc.compile()
    return nc


# ----------------------------------------------------------------------------
# Entry point
# ----------------------------------------------------------------------------
def _prepare(feats, pos, filt, dense_w, dense_b, src_idx, qry_idx):
    feats = np.ascontiguousarray(np.asarray(feats, np.float32))
    pos = np.ascontiguousarray(np.asarray(pos, np.float32))
    filt = np.asarray(filt, np.float32)
    dense_w = np.asarray(dense_w, np.float32)
    dense_b = np.asarray(dense_b, np.float32)
    src_idx = np.asarray(src_idx).astype(np.int64)
    qry_idx = np.asarray(qry_idx).astype(np.int64)

    plans, bstart, bsz = _plan(qry_idx)
    NCH = max(len(p['chunks']) for p in plans)
    NCHP = ((NCH + 15) // 16) * 16
    NQ = NCHP * 16

    feats_bf = feats.astype(BF16)

    # filter regroup: G2[ax*32+c, t*64+o] = filt[az, ay, ax, c, o], t = az*4+ay
    G2 = np.zeros((128, 16 * 64), np.float32)
    for az in range(4):
        for ay in range(4):
            t = az * 4 + ay
            for ax in range(4):
                G2[ax * 32:(ax + 1) * 32, t * 64:(t + 1) * 64] = filt[az, ay, ax]

    in_maps = []
    for c, p in enumerate(plans):
        possrc, posqry, fsrc, sidx = _pack_core(p, bstart, pos, feats_bf,
                                                qry_idx, src_idx, NCHP)
        ftT = np.zeros((CIN, NQ), BF16)
        ftT[:, 0:p['nq']] = feats_bf[p['q0']:p['q0'] + p['nq']].T
        in_maps.append({
            "possrc": possrc, "posqry": posqry, "fsrc": fsrc, "sidx": sidx,
            "g2": G2, "featsT": ftT, "denw": dense_w.astype(BF16),
            "denb": dense_b.reshape(COUT, 1).astype(np.float32),
        })

    key = (NCHP, NQ)
    if key not in _COMPILED:
        _COMPILED[key] = _build_bass(NCHP, NQ)
    nc = _COMPILED[key]
    return nc, in_maps, plans


def kernel(feats, pos, filt, dense_w, dense_b, src_idx, qry_idx):
    from concourse.bass_utils import run_bass_kernel_spmd

    nc, in_maps, plans = _prepare(feats, pos, filt, dense_w, dense_b,
                                  src_idx, qry_idx)
    res = run_bass_kernel_spmd(nc, in_maps, core_ids=list(range(NCORES)))

    ans_conv = np.zeros((N, COUT), np.float32)
    ans_dense = np.zeros((N, COUT), np.float32)
    for c, p in enumerate(plans):
        outT = res.results[c]["outconvT"]
        for ci, (bA, bB) in enumerate(p['chunks']):
            for half, b in enumerate((bA, bB)):
                if b is None:
                    continue
                cols = ci * 16 + half * 8
                ans_conv[8 * b:8 * b + 8] = outT[:, cols:cols + 8].T
        dT = res.results[c]["outdenseT"]
        ans_dense[p['q0']:p['q0'] + p['nq']] = dT[:, 0:p['nq']].T
    return ans_conv, ans_dense
